# revision 27
# baseline (speedup 1.0000x reference)
"""Trainium2 Bass kernel for single-head causal attention.

Problem: B=8, T=2048, C=1024, HS=64 (data-parallel: one batch element
per NeuronCore, no collectives).
  q = x_q @ Wq; k = x_kv @ Wk; v = x_kv @ Wv
  wei = softmax(mask(q @ k.T * C**-0.5));  out = wei @ v

Design (v3, tuned against the TimelineSim cost model):
  - 8 slices of TS=256: the input DMA stream (xq fp8 2MB + xk fp16
    4MB ~ 18.6us at 360GB/s serial) paces the kernel; fine slices
    start compute at ~5us and keep PE/Act fed as data lands.
  - ALL input DMAs on the SP ring in exact consumption order
    (weights first).  The 8 round-robin DMAHW completion sems then
    chain bulk loads onto early transfers only, and the Act/DVE/Pool
    sequencers stay DMA-free.
  - x_q/Wq fp8(e4m3), host-quantized, W pre-scaled by 16; q-proj uses
    DoubleRow (2x).  x_kv/Wkv fp16 (v inherits projection error in
    full; fp8 costs ~4e-2 rel err vs the 2e-2 budget).
  - exp on Act is the steady-state pacer (~0.83ns/col + ~185ns/instr
    overhead): scores are batched 4 k-blocks per exp instruction; the
    2 diagonal blocks (compacted to 384 cols) + odd leftover pair
    ride in one batch.
  - no row-tiling / kT shift: the cost model treats PE as serial, and
    the shift chain delayed the first exp.
  - v_sb built by PE-transpose into the dead c_ps PSUM + Pool copies
    (no DMA: HWDGE issue slots ~625ns each are a second serial
    resource).
  - PV: fp16, v padded to 96 rows; col 64 = 16.0 supplies the softmax
    denominator through the same matmuls.
  - finalize per tile: Pool copy of o_ps, 4 PE transposes, reciprocal
    + broadcast mul into a persistent ostage; 2 output DMAs total
    (tiles 0-5 during the post-stream DMA idle window, 6-7 at tail).
  - tile 7 reordered: full blocks (k slices 0-6, q7) first, kv-proj 7
    + diagonal last, so the final xk transfer (~20.7us) gates only a
    ~2us tail.
"""

import sys

sys.path.insert(0, "/opt/trn_rl_repo")

import numpy as np
import ml_dtypes

import concourse.bass as bass
from concourse import bacc
import concourse.mybir as mybir
import concourse.tile as tile
from concourse.bass_utils import run_bass_kernel_spmd
from concourse.masks import make_identity

FP32 = mybir.dt.float32
F16 = mybir.dt.float16
F8 = mybir.dt.float8e4

T, C, HS = 2048, 1024, 64
NSLICE = 8
TS = T // NSLICE          # 256
CK = C // 128             # 8
NJ = T // 128             # 16
MP = 96                   # padded PV out rows
WS = 16.0                 # host weight scale
SCALE = float(C) ** -0.5 / (WS * WS)
DR = mybir.MatmulPerfMode.DoubleRow
EXPF = mybir.ActivationFunctionType.Exp


def build_bass(reps=1):
    nc = bacc.Bacc(None, target_bir_lowering=False)
    # partition-major host layout: per-partition runs are CK*TS bytes
    # contiguous, so DMA elem >= 2KB (sub-512B elems pay a 2x penalty)
    xq = nc.dram_tensor("xq", [NSLICE, 128, CK, TS], F8, kind="ExternalInput").ap()
    xk = nc.dram_tensor("xk", [NSLICE, 128, CK, TS], F16, kind="ExternalInput").ap()
    wq = nc.dram_tensor("wq", [128, CK, 64], F8, kind="ExternalInput").ap()
    wkv = nc.dram_tensor("wkv", [128, CK, 128], F16, kind="ExternalInput").ap()
    out = nc.dram_tensor("out", [T, HS], F16, kind="ExternalOutput").ap()

    with tile.TileContext(nc) as tc:
        with (
            tc.tile_pool(name="singles", bufs=1) as singles,
            tc.tile_pool(name="xp", bufs=1) as xp,
            tc.tile_pool(name="proj", bufs=1) as proj,
            tc.tile_pool(name="pstage", bufs=8) as pstage,
            tc.tile_pool(name="fin", bufs=2) as fin,
            tc.tile_pool(name="ost", bufs=1) as ost,
            tc.tile_pool(name="pp_c", bufs=1, space="PSUM") as pp_c,
            tc.tile_pool(name="pp_st", bufs=2, space="PSUM") as pp_st,
            tc.tile_pool(name="pp_o", bufs=3, space="PSUM") as pp_o,
        ):
            xq_sb = xp.tile([128, NSLICE, CK, TS], F8)
            xk_sb = xp.tile([128, NSLICE, CK, TS], F16)
            wq_sb = singles.tile([128, CK, 64], F8)
            wkv_sb = singles.tile([128, CK, 128], F16)

            def emit_x_dmas():
                nc.sync.dma_start(out=wq_sb, in_=wq)
                nc.sync.dma_start(out=xq_sb[:, 0], in_=xq[0])
                nc.sync.dma_start(out=wkv_sb, in_=wkv)
                nc.sync.dma_start(out=xk_sb[:, 0], in_=xk[0])
                for s in range(1, NSLICE):
                    nc.sync.dma_start(out=xq_sb[:, s], in_=xq[s])
                    nc.sync.dma_start(out=xk_sb[:, s], in_=xk[s])

            # ---- one-time constants (overlap the DMA latency) ----
            ident = singles.tile([128, 128], F16)
            make_identity(nc, ident)
            tri = singles.tile([128, 128], F16)
            nc.gpsimd.memset(tri, 1.0)
            nc.gpsimd.affine_select(
                out=tri,
                in_=tri,
                compare_op=mybir.AluOpType.is_ge,
                fill=0.0,
                base=0,
                pattern=[[1, 128]],
                channel_multiplier=-1,
            )

            # ---- persistent activations ----
            # comb planes: 0 = kvT (k rows 0:64, v rows 64:128), 1 = qT
            comb = proj.tile([128, 2, T], F16)
            v_sb = proj.tile([128, NJ, MP], F16)
            nc.gpsimd.memset(v_sb[:, :, HS : HS + 1], WS)
            nc.gpsimd.memset(v_sb[:, :, HS + 1 : MP], 0.0)
            ostage = ost.tile([64, NSLICE, 4, HS], F16)

            cur_c = [None]

            def emit_qproj(s):
                cur_c[0] = pp_c.tile([128, 2, TS], FP32, tag="c", name="c_ps")
                c = cur_c[0]
                for ci in range(CK // 2):
                    nc.tensor.matmul(
                        c[0:64, 1, :],
                        wq_sb[:, 2 * ci : 2 * ci + 2, :],
                        xq_sb[:, s, 2 * ci : 2 * ci + 2, :],
                        start=(ci == 0),
                        stop=(ci == CK // 2 - 1),
                        perf_mode=DR,
                        skip_group_check=True,
                    )
                t0 = s * TS
                nc.vector.tensor_copy(comb[0:64, 1:2, t0 : t0 + TS], c[0:64, 1:2, :])

            def emit_kvproj(s):
                c = cur_c[0]
                for ci in range(CK):
                    nc.tensor.matmul(
                        c[:, 0, :],
                        wkv_sb[:, ci, :],
                        xk_sb[:, s, ci, :],
                        start=(ci == 0),
                        stop=(ci == CK - 1),
                        skip_group_check=True,
                    )
                t0 = s * TS
                nc.vector.tensor_copy(comb[:, 0:1, t0 : t0 + TS], c[:, 0:1, :])

            def emit_vt(s):
                # vT -> natural v layout via PE transpose (no DMA: HWDGE
                # issue slots are a second serial resource), Pool copies out.
                # Emitted AFTER other PE work so the comb copy (DVE) it
                # waits on has drained -- keeps the in-order PE stream busy.
                t0 = s * TS
                vt = pp_o.tile([128, 128], F16, tag="o", name="vt")
                for b in (0, 1):
                    nc.tensor.transpose(
                        vt[:, 64 * b : 64 * b + 64],
                        comb[64:128, 0, t0 + 128 * b : t0 + 128 * (b + 1)],
                        ident[64:128, 64:128],
                    )
                    # GPSIMD cannot access PSUM: DVE does the copy-out
                    nc.vector.tensor_copy(
                        v_sb[:, 2 * s + b, 0:HS], vt[:, 64 * b : 64 * b + 64]
                    )

            def kblk(j):
                return comb[0:64, 0, j * 128 : (j + 1) * 128]

            def qmov(i, c0=0, c1=TS):
                t0 = i * TS
                return comb[0:64, 1, t0 + c0 : t0 + c1]

            def emit_scores_batch(i, blocks):
                """Full k-blocks `blocks` vs all TS queries of tile i:
                one st tile + one exp, 256 cols per block."""
                st = pp_st.tile([128, 1024], FP32, tag="st", name="st")
                for n, j in enumerate(blocks):
                    nc.tensor.matmul(
                        st[:, n * TS : (n + 1) * TS],
                        kblk(j),
                        qmov(i),
                        start=True,
                        stop=True,
                    )
                p = pstage.tile([128, 1024], F16, tag="p", name="p")
                ncols = len(blocks) * TS
                nc.scalar.activation(
                    out=p[:, 0:ncols], in_=st[:, 0:ncols], func=EXPF, scale=SCALE
                )
                return p

            def emit_diag_batch(i, lblocks):
                """Leftover full blocks + the 2 compacted diagonal blocks
                (m0: 256 cols, m1: last 128 queries) in one exp."""
                jd = 2 * i
                L = len(lblocks)
                st = pp_st.tile([128, 1024], FP32, tag="st", name="st")
                for n, j in enumerate(lblocks):
                    nc.tensor.matmul(
                        st[:, n * TS : (n + 1) * TS],
                        kblk(j),
                        qmov(i),
                        start=True,
                        stop=True,
                    )
                o0 = L * TS
                nc.tensor.matmul(
                    st[:, o0 : o0 + TS], kblk(jd), qmov(i), start=True, stop=True
                )
                nc.tensor.matmul(
                    st[:, o0 + TS : o0 + TS + 128],
                    kblk(jd + 1),
                    qmov(i, 128, TS),
                    start=True,
                    stop=True,
                )
                p = pstage.tile([128, 1024], F16, tag="p", name="p")
                ncols = o0 + TS + 128
                nc.scalar.activation(
                    out=p[:, 0:ncols], in_=st[:, 0:ncols], func=EXPF, scale=SCALE
                )
                return p

            def emit_tri(p, L, dve_only=False):
                o0 = L * TS
                eng2 = nc.vector if dve_only else nc.gpsimd
                nc.vector.tensor_mul(p[:, o0 : o0 + 128], p[:, o0 : o0 + 128], tri)
                eng2.tensor_mul(
                    p[:, o0 + TS : o0 + TS + 128], p[:, o0 + TS : o0 + TS + 128], tri
                )

            def emit_pv_diag(o_ps, i, pD, lblocks, first, last):
                jd = 2 * i
                L = len(lblocks)
                for n, j in enumerate(lblocks):
                    nc.tensor.matmul(
                        o_ps[0:MP, :],
                        v_sb[:, j, :],
                        pD[:, n * TS : (n + 1) * TS],
                        start=(first and n == 0),
                        stop=False,
                        skip_group_check=True,
                    )
                o0 = L * TS
                nc.tensor.matmul(
                    o_ps[0:MP, :],
                    v_sb[:, jd, :],
                    pD[:, o0 : o0 + TS],
                    start=(first and L == 0),
                    stop=False,
                    skip_group_check=True,
                )
                nc.tensor.matmul(
                    o_ps[0:MP, 128:TS],
                    v_sb[:, jd + 1, :],
                    pD[:, o0 + TS : o0 + TS + 128],
                    start=False,
                    stop=last,
                    skip_group_check=True,
                )

            def emit_pv_batch(o_ps, p, blocks, first, last):
                for n, j in enumerate(blocks):
                    nc.tensor.matmul(
                        o_ps[0:MP, :],
                        v_sb[:, j, :],
                        p[:, n * TS : (n + 1) * TS],
                        start=(first and n == 0),
                        stop=(last and n == len(blocks) - 1),
                        skip_group_check=True,
                    )

            def emit_fin_copy(i, o_ps, on_dve):
                ot = fin.tile([MP, TS], F16, tag="ot", name="ot")
                nc.vector.tensor_copy(ot, o_ps[0:MP, :])
                return ot

            def emit_fin_rest(i, ot, on_dve):
                ft = pp_o.tile([64, 4, MP], F16, tag="o", name="ft")
                for u in range(4):
                    nc.tensor.transpose(
                        ft[:, u, :], ot[:, 64 * u : 64 * u + 64], ident[0:MP, 0:MP]
                    )
                rec = fin.tile([64, 4, 1], FP32, tag="rec", name="rec")
                nc.vector.reciprocal(rec, ft[:, :, HS : HS + 1])
                nc.vector.tensor_mul(
                    ostage[:, i], ft[:, :, 0:HS], rec.broadcast_to((64, 4, HS))
                )

            def emit_finalize(i, o_ps, on_dve):
                emit_fin_rest(i, emit_fin_copy(i, o_ps, on_dve), on_dve)

            for rep in range(reps):
                emit_x_dmas()
                def tparams(i):
                    nf = 2 * i
                    nb4 = nf // 4
                    return nb4, list(range(4 * nb4, nf))

                def emit_scores(i, with_diag=True):
                    nb4, lblocks = tparams(i)
                    pD = emit_diag_batch(i, lblocks) if with_diag else None
                    pbs = []
                    for b in range(nb4):
                        pbs.append(
                            (emit_scores_batch(i, range(4 * b, 4 * b + 4)),
                             list(range(4 * b, 4 * b + 4)))
                        )
                    return pD, pbs

                # prologue: slice 0 projection + tile 0 scores
                emit_qproj(0)
                emit_kvproj(0)
                emit_vt(0)
                cur = emit_scores(0)
                prev_fin = None  # (tile, ot, on_dve) awaiting ft/recip/mul

                for i in range(NSLICE):
                    nb4, lblocks = tparams(i)
                    L = len(lblocks)
                    pD, pbs = cur
                    last = i == NSLICE - 1
                    o_ps = pp_o.tile([MP, TS], FP32, tag="o", name="o_ps")

                    if prev_fin is not None:
                        # o_ps(i-1) -> SBUF early (Pool): the ft transposes
                        # at this iteration's end see it completed
                        fi, fo = prev_fin
                        ot_prev = emit_fin_copy(fi, fo, on_dve=False)

                    if not last:
                        # PVs interleave with proj(i+1): each fills the
                        # other's dependency latencies in the in-order PE
                        # stream (xk arrival / comb-copy drain / exp drain)
                        emit_tri(pD, L)
                        emit_pv_diag(o_ps, i, pD, lblocks, True, nb4 == 0)
                        emit_qproj(i + 1)
                        if nb4 >= 1:
                            emit_pv_batch(o_ps, pbs[0][0], pbs[0][1],
                                          first=False, last=(nb4 == 1))
                        if i < NSLICE - 2:
                            emit_kvproj(i + 1)
                        for n in range(1, nb4):
                            emit_pv_batch(o_ps, pbs[n][0], pbs[n][1],
                                          first=False, last=(n == nb4 - 1))
                        if i < NSLICE - 2:
                            emit_vt(i + 1)
                            nxt = emit_scores(i + 1)
                        else:
                            # tile 7: fulls only now; kv-proj(7) + diag
                            # ride inside iteration 7
                            nxt = (None, emit_scores(i + 1, with_diag=False)[1])
                    else:
                        # tile 7: kv-proj(7) first (xk7-gated), full-batch
                        # PVs fill its latency, diagonal last
                        emit_kvproj(i)
                        for n in range(nb4):
                            emit_pv_batch(o_ps, pbs[n][0], pbs[n][1],
                                          first=(n == 0), last=False)
                        emit_vt(i)
                        pD = emit_diag_batch(i, lblocks)
                        emit_tri(pD, L, dve_only=True)
                        emit_pv_diag(o_ps, i, pD, lblocks, False, True)

                    if prev_fin is not None:
                        emit_fin_rest(prev_fin[0], ot_prev, on_dve=False)
                        if prev_fin[0] in (5, 6):
                            # flush staged tiles in the post-stream DMA idle
                            # window; only tile 7 (32KB) rides the tail
                            r0 = 0 if prev_fin[0] == 5 else 6 * TS
                            s0 = 0 if prev_fin[0] == 5 else 6
                            nc.sync.dma_start(
                                out=out[r0 : (prev_fin[0] + 1) * TS].rearrange(
                                    "(i u p) h -> p i u h", u=4, p=64
                                ),
                                in_=ostage[:, s0 : prev_fin[0] + 1],
                            )
                    if last:
                        emit_finalize(i, o_ps, on_dve=True)
                    else:
                        prev_fin = (i, o_ps)
                        cur = nxt
                nc.sync.dma_start(
                    out=out[7 * TS : T].rearrange("(i u p) h -> p i u h", u=4, p=64),
                    in_=ostage[:, 7:8],
                )
    nc.compile()
    return nc


_NC_CACHE = {}


def _get_nc(reps=1):
    if reps not in _NC_CACHE:
        _NC_CACHE[reps] = build_bass(reps)
    return _NC_CACHE[reps]


def make_inputs(x_q, x_kv, Wq, Wk, Wv):
    """Host-side prep: quantize + lay out inputs per core (batch b)."""

    def _swz(w, dt):  # [C, N] -> [128, CK, N]
        n = w.shape[1]
        return np.ascontiguousarray(
            w.reshape(CK, 128, n).transpose(1, 0, 2)
        ).astype(dt)

    wq_h = _swz(Wq * WS, ml_dtypes.float8_e4m3)
    wkv_h = _swz(np.concatenate([Wk, Wv], axis=1) * WS, np.float16)
    B = x_q.shape[0]
    def _xswz(x, dt):  # [B, T, C] -> [B, NSLICE, 128, CK, TS]
        a = x.transpose(0, 2, 1).reshape(B, CK, 128, NSLICE, TS)
        return np.ascontiguousarray(a.transpose(0, 3, 2, 1, 4)).astype(dt)

    xqT = _xswz(x_q, ml_dtypes.float8_e4m3)
    xkT = _xswz(x_kv, np.float16)
    return [
        {"xq": xqT[b], "xk": xkT[b], "wq": wq_h, "wkv": wkv_h} for b in range(B)
    ]


def kernel(x_q, x_kv, Wq, Wk, Wv, _trace=False, _reps=1):
    B = x_q.shape[0]
    assert B == 8 and x_q.shape == (8, T, C)
    in_maps = make_inputs(x_q, x_kv, Wq, Wk, Wv)
    nc = _get_nc(_reps)
    res = run_bass_kernel_spmd(nc, in_maps, core_ids=list(range(B)), trace=_trace)
    out = np.stack([r["out"].astype(np.float32) for r in res.results])
    if _trace:
        kernel.last_result = res
    return out
